# revision 88
# baseline (speedup 1.0000x reference)
"""Trainium2 Bass kernel for nn_AllPassMORRCirculantConv2d.

Math: out[n, (pp,t)] = sum_q scale_q * tr(phase[n,pp,q,t]) with
  tr(x) = (A^2 - 2*rho*cos x + R^2) / (1 - 2*rho*cos x + rho^2),  rho = A*R.
Since sum_q scale_q = 0 (differential rails), this reduces to
  out = -(C1/(2*rho)) * sum_q scale_q / (K - cos(phase_q)),
  C1 = (1-A^2)(1-R^2),  K = (1+rho^2)/(2*rho).

Device pipeline per 512-pixel tile and group of 3 q-pairs:
  PE  : v = phase/2pi via block-circulant matmul into PSUM
        (|w|/2pi circulant blocks built on host; K=32 blocks zero-padded so
        every rhs slice is a 32-aligned row-group of the rep slab)
  DVE : u = |v - rtne(v)| in ONE custom op (FRAC_ABS_MORR: the +MAGIC/-MAGIC
        round-to-nearest trick runs on the DVE's own fp32 adders)
  ACT : c = Sin(-2pi*u + pi/2) = cos(phase)   (u in [0,1/2] -> arg in
        [-pi/2, pi/2], safely inside the Sin LUT's valid range)
  DVE : r ~= 1/(K - c) in ONE custom op (RECIP_KSUB_MORR: K-subtract +
        bitwise-NOT seed + 1 Newton step, ~1.7e-3 rel, bf16 output)
  PE  : out_psum += S_j^T @ r   (bf16 q-reduction with host-built
        scale-diagonal; accumulated over all 18 q-pairs in PSUM)

The 3x3 unfold is done once per core: square+zero-pad x into a [32, 66*66]
SBUF plane, then write 288 contiguous 64*66-element "slabs" (row w = 9c+k
= channel c shifted by (kh,kw)) to a DRAM scratch; per-pixel-tile loads pull
32-aligned row blocks back and the matmul rhs AP walks the 66-stride window.

Data-parallel over the batch: core b handles image b. No collectives.
Engine budget per core (cost model): DVE ~182us (bottleneck: 2 elementwise
passes over the 9.4M-element phase tensor), ACT ~100us, PE ~160us cold /
~95us warm, DMA ~105us; total ~207us.
"""

import sys

for _p in ("/opt/trn_rl_repo", "/opt/pypackages"):
    if _p not in sys.path:
        sys.path.insert(0, _p)

import numpy as np

# --- problem constants (hardcoded; kernel.py must be self-contained) ---
A_ = 0.987
R_ = 0.99
RHO = A_ * R_
C1TR = (1.0 - A_ * A_) * (1.0 - R_ * R_)
C2TR = 1.0 + RHO * RHO
KCONST = C2TR / (2.0 * RHO)          # ~1.000268
BETA = -C1TR / (2.0 * RHO)           # folded into the q-reduction weights
CSC = 1.0 / (2.0 * np.pi)            # folded into the circulant weights
MAGIC = float(np.float32(1.5 * 2 ** 23))

B_, IN_C, H_, W_ = 8, 32, 64, 64
OUT_C, MB = 64, 8
Q_, P_ = 36, 8                        # grid dims; wic = 288
NPAIR = Q_ // 2                       # 18 q-pairs -> K=16 matmuls
L_ = H_ * W_                          # 4096 pixels per image
TPIX = 512                            # pixels per tile (fp32 matmul N max)
NTILE = L_ // TPIX

# tuning knobs (validated via TimelineSim)
GROUP_PAIRS = 3                       # q-pairs per ACT/DVE op (FD = 512*GROUP_PAIRS)
VBUFS = 2                             # PSUM v-tile buffers
WBUFS = 2                             # SBUF work-tile buffers


def host_prep(weight: np.ndarray, morr_output_scale: np.ndarray):
    """Build the two small replicated device matrices from the raw params.

    lhsT [16, 18*128]: block-circulant |w| * (1/2pi) for the phase matmul.
      lhsT[h*8+s, j*128 + h*64 + pp*8 + t] = CSC * |w|[pp, 2j+h, (t-s) % 8]
    smat [128, 18*64]: q-reduction diagonal-ish weights.
      smat[h*64+u, j*64 + u] = BETA * scalevec[2j+h]
    """
    wabs = np.abs(weight.astype(np.float64))                    # [8, 36, 8]
    s_i = np.arange(8)
    idx = (s_i[None, :] - s_i[:, None]) % 8                     # [s, t] -> (t-s)%8
    circ = wabs[:, :, idx]                                      # [pp, q, s, t]
    # K=32 blocks: pair j contracts rep rows [32m, 32m+32), m = j//2, placed
    # at PE array row-group 32*(m%4) (m=8 -> 0). lhsT block j is stored at
    # the matching partition rows so ldweights/rhs bases line up.
    lhsT = np.zeros((128, NPAIR, 128), np.float64)
    for j in range(NPAIR):
        m = j // 2
        prow = 32 * (m % 4) if m < 8 else 0
        ro = prow + (j % 2) * 16
        for h in range(2):
            q = 2 * j + h
            for pp in range(P_):
                lhsT[
                    ro + h * 8: ro + h * 8 + 8,
                    j,
                    h * 64 + pp * 8: h * 64 + pp * 8 + 8,
                ] = CSC * circ[pp, q]
    lhsT = np.ascontiguousarray(lhsT.reshape(128, NPAIR * 128).astype(np.float32))

    sv = morr_output_scale.astype(np.float64)
    scalevec = np.concatenate([sv[:-1], -sv[:-1]])              # [36], q even branch
    smat = np.zeros((128, NPAIR, 64), np.float64)
    u_i = np.arange(64)
    for j in range(NPAIR):
        for h in range(2):
            smat[h * 64 + u_i, j, u_i] = BETA * scalevec[2 * j + h]
    import ml_dtypes

    smat = np.ascontiguousarray(
        smat.reshape(128, NPAIR * 64).astype(ml_dtypes.bfloat16)
    )
    return lhsT, smat


def host_unfold_sq(x_img: np.ndarray) -> np.ndarray:
    """numpy mirror of the on-device unfold+square: rep[w, l] (mini-model only)."""
    xsq = np.zeros((IN_C, 66, 66), np.float32)
    xsq[:, 1:65, 1:65] = (x_img * x_img).astype(np.float32)
    rep = np.zeros((288, L_), np.float32)
    for c in range(IN_C):
        for kh in range(3):
            for kw in range(3):
                w = c * 9 + kh * 3 + kw
                rep[w] = xsq[c, kh:kh + 64, kw:kw + 64].reshape(-1)
    return rep


def minimodel(x_img, lhsT, smat):
    """Pure-numpy fp32 mirror of the device pipeline for one image (debugging)."""
    rep = host_unfold_sq(x_img)
    out = np.zeros((64, L_), np.float32)
    for j in range(NPAIR):
        m = j // 2
        prow = 32 * (m % 4) if m < 8 else 0
        lh = lhsT[prow:prow + 32, j * 128:(j + 1) * 128]        # [32, 128]
        rhs = rep[32 * m:32 * m + 32, :]                        # [32, L]
        v = (lh.T.astype(np.float32) @ rhs).astype(np.float32)  # [128, L]
        t = np.float32(v + np.float32(MAGIC))
        u = np.abs(np.float32(v - np.float32(t - np.float32(MAGIC))))
        arg = np.float32(u * np.float32(-2 * np.pi) + np.float32(0.5 * np.pi))
        assert np.abs(arg).max() <= np.pi, "sin range"
        c = np.sin(arg, dtype=np.float32)                       # = cos(phase)
        # device RECIP_KSUB_MORR: seed + 1 NR, ~1.7e-3 rel
        d = np.float32(np.float32(KCONST) - c)
        nd = (~d.view(np.int32)).view(np.float32)
        y0 = np.float32(np.float32(RECIP_C0) * nd)
        r = np.float32(y0 * np.float32(np.float32(RECIP_C1) - d * y0))
        import ml_dtypes

        sm = smat[:, j * 64:(j + 1) * 64].astype(np.float32)    # [128, 64] (bf16)
        rb = r.astype(ml_dtypes.bfloat16).astype(np.float32)
        out += (sm.T @ rb).astype(np.float32)
    return out


# ----------------------------------------------------------------------------
# custom DVE ops
# ----------------------------------------------------------------------------

_CUSTOM = {}


def _register_custom_ops():
    """Register two fused custom DVE ops (appended to the dve_ops registry):

    FRAC_ABS_MORR: out = |in0 - ((in0 + s0) - s0)|    (3 ALU stages)
      -> u = |v - round(v)| in one pass; the DVE fp32 ALU adds perform the
      round-to-nearest-even magic-constant trick (s0 = 1.5*2^23).
    RECIP_KSUB_MORR: out ~= 1/(imm2 - in0), bitwise-NOT seed + 1 Newton step
      (6 stages, ~1.7e-3 rel err) -> r = 1/(K - cos phi) in one pass.
    """
    if _CUSTOM:
        return _CUSTOM
    from concourse import dve_ops
    from concourse.dve_spec import Spec, Bin, AluOp, Src0, Src1, C0, C1, C2, lower
    from concourse.dve_spec import _has_src1
    from concourse.dve_uop import DveOpSpec

    def make(name, spec, subdim=False):
        row = max(dve_ops._SUB_OPCODE_FOR_NAME.values()) + 1
        dve_ops._SUB_OPCODE_FOR_NAME[name] = row
        shas = {}
        for ver in ("v3", "v4"):
            try:
                ds = DveOpSpec(
                    name=name,
                    opcode=row,
                    uops=lower(spec, ver=ver),
                    rd1_en=_has_src1(spec),
                )
                shas[ver] = ds.sha(ver)
            except Exception:
                pass
        op = dve_ops.DveOp(name, spec, subdim=subdim, uops_sha=shas)
        dve_ops.OPS.append(op)
        dve_ops.CUSTOM_DVE_SPECS[name] = spec
        return op

    _t = Src0 + C0
    _k = _t - C0

    def _ref_frac_abs(in0, in1, s0, s1, imm2):
        t = (in0 + np.float32(s0)).astype(np.float32)
        k = (t - np.float32(s0)).astype(np.float32)
        return np.abs(in0 - k).astype(np.float32)

    frac_abs = make(
        "FRAC_ABS_MORR",
        Spec(
            body=Bin(AluOp.ABSOLUTE_DIFF, Src0, _k),
            reference=_ref_frac_abs,
        ),
    )

    _d = C2 - Src0
    _nd = Bin(AluOp.BITWISE_NOT, _d, _d)
    _y0 = _nd * C0

    def _ref_recip_ksub(in0, in1, s0, s1, imm2):
        d = (np.float32(imm2) - in0).astype(np.float32)
        nd = (~d.view(np.int32)).view(np.float32)
        y0 = (np.float32(s0) * nd).astype(np.float32)
        return (y0 * (np.float32(s1) - d * y0)).astype(np.float32)

    recip_ksub = make(
        "RECIP_KSUB_MORR",
        Spec(body=_y0 * (C1 - _d * _y0), reference=_ref_recip_ksub),
    )
    _CUSTOM["frac_abs"] = frac_abs
    _CUSTOM["recip_ksub"] = recip_ksub
    return _CUSTOM


RECIP_C0 = -0.23549792
RECIP_C1 = 2.0017324


# ----------------------------------------------------------------------------
# device kernel build
# ----------------------------------------------------------------------------

def _build(tc, dram):
    from contextlib import ExitStack
    import concourse.mybir as mybir

    ctx = ExitStack()
    nc = tc.nc
    F32 = mybir.dt.float32
    AF = mybir.ActivationFunctionType
    ops = _register_custom_ops()

    from concourse.bass_types import AP as BAP

    BF16 = mybir.dt.bfloat16
    cpool = ctx.enter_context(tc.tile_pool(name="const", bufs=1))
    vpool = ctx.enter_context(tc.tile_pool(name="vps", bufs=VBUFS, space="PSUM"))
    opool = ctx.enter_context(tc.tile_pool(name="ops", bufs=2, space="PSUM"))
    wpool = ctx.enter_context(tc.tile_pool(name="work", bufs=WBUFS))
    outp = ctx.enter_context(tc.tile_pool(name="outsb", bufs=2))
    rpool = ctx.enter_context(tc.tile_pool(name="repsb", bufs=4))
    x_sb = cpool.tile([IN_C, L_], F32)
    XSPLIT = 35 * W_  # rows 0..34 feed the first unfold chunk
    nc.sync.dma_start(x_sb[:, :XSPLIT], dram["x"][:, :XSPLIT])
    nc.sync.dma_start(x_sb[:, XSPLIT:], dram["x"][:, XSPLIT:])

    # per-partition constant columns for activation scale/bias operands
    cst_n2pi = cpool.tile([128, 1], F32)
    nc.gpsimd.memset(cst_n2pi[:], float(np.float32(-2.0 * np.pi)))
    cst_hpi = cpool.tile([128, 1], F32)
    nc.gpsimd.memset(cst_hpi[:], float(np.float32(0.5 * np.pi)))

    # dummy Sin to pull the trig table load (~2.7us) into the prologue where
    # it overlaps the input DMAs instead of stalling the first real sin
    warm = cpool.tile([1, 8], F32)
    nc.vector.memset(warm[:], 0.0)
    nc.scalar.activation(warm[:], warm[:], AF.Sin)

    # PE p-state warmup: chained dummy matmuls keep the PE busy through the
    # prologue so the first real matmuls run fully ramped.
    win = cpool.tile([1, TPIX], F32)
    nc.gpsimd.memset(win[:], 0.0)
    vdum = vpool.tile([128, GROUP_PAIRS * TPIX], F32, name="v_ps", tag="v_ps")
    for _ in range(10):
        nc.tensor.matmul(
            vdum[:1, :128], win[:1, :1], win[:1, :128], start=True, stop=True
        )

    # squared, zero-padded input plane: [32, 66*66 (+4 spare for slab reads)].
    # Only the border strips need zeroing; the squares fill the interior.
    xsq = cpool.tile([IN_C, 66 * 66 + 4], F32)
    x3 = x_sb[:].rearrange("p (a b) -> p a b", b=W_)
    xq3 = xsq[:, :66 * 66].rearrange("p (a b) -> p a b", b=66)
    nc.gpsimd.memset(xsq[:, 0:66], 0.0)                     # top pad row
    nc.gpsimd.memset(xsq[:, 65 * 66:66 * 66 + 4], 0.0)      # bottom row + spare
    nc.gpsimd.memset(xq3[:, 1:65, 0:1], 0.0)                # left pad col
    nc.gpsimd.memset(xq3[:, 1:65, 65:66], 0.0)              # right pad col
    nc.scalar.activation(xq3[:, 1:36, 1:65], x3[:, 0:35], AF.Square)
    nc.scalar.activation(xq3[:, 36:65, 1:65], x3[:, 35:64], AF.Square)

    # unfold replication via DRAM slabs: rep row w = 9c + kappa holds the
    # CONTIGUOUS 64*66-element slab of channel c's padded plane starting at
    # (kh*66 + kw). Two column-chunk tensors isolate the dependency so the
    # first pixel tiles start after chunk 0 only; kw is folded into the DMA
    # AP so each chunk is 3 DMAs (one per kh).
    # unfold chunk writes interleaved with the matching slab loads so the
    # serial DMA path produces TT=0's inputs first.
    SLAB = 64 * 66
    CHUNK = 16 * 66
    RL = 16 * 66
    xpitch = xsq[:].ap[0][0]
    repAs, repBs, repCs = [], [], []
    lh_sb = sm_sb = None
    for q4 in range(4):
        repc = dram[f"repd{q4}"]
        dpitch = repc.ap[0][0]
        cs = q4 * CHUNK
        for kh in range(3):
            src = BAP(
                tensor=xsq[:].tensor,
                offset=xsq[:].offset + kh * 66 + cs,
                ap=[[xpitch, IN_C], [1, 3], [1, CHUNK]],
            )
            dst = BAP(
                tensor=repc.tensor,
                offset=3 * kh * dpitch,
                ap=[[9 * dpitch, IN_C], [dpitch, 3], [1, CHUNK]],
            )
            nc.sync.dma_start(dst, src)
        repA = rpool.tile([128, RL], F32, name=f"repA{q4}", tag="repA")
        nc.sync.dma_start(repA[:], repc[0:128, :])
        repB = rpool.tile([128, RL], F32, name=f"repB{q4}", tag="repB")
        nc.sync.dma_start(repB[:], repc[128:256, :])
        repC = rpool.tile([32, RL], F32, name=f"repC{q4}", tag="repC")
        nc.sync.dma_start(repC[:], repc[256:288, :])
        repAs.append(repA)
        repBs.append(repB)
        repCs.append(repC)
        if q4 == 0:
            # weights: needed by the first matmuls; right after TT=0's slabs.
            # The first group's 3 blocks load first (tiny) so MM g=0 starts
            # without waiting for the full 1.2 MB weight tensor.
            W0 = GROUP_PAIRS * 128
            lh_sb = cpool.tile([128, NPAIR * 128], F32)
            nc.sync.dma_start(lh_sb[:, :W0], dram["lhsT"][:, :W0])
            nc.sync.dma_start(lh_sb[:, W0:], dram["lhsT"][:, W0:])
            sm_sb = cpool.tile([128, NPAIR * 64], BF16)
            nc.sync.dma_start(sm_sb[:], dram["smat"][:])

    groups = []
    jj = 0
    while jj < NPAIR:
        n = min(GROUP_PAIRS, NPAIR - jj)
        groups.append(list(range(jj, jj + n)))
        jj += n

    for TT in range(NTILE // 2):
        repA, repB, repC = repAs[TT], repBs[TT], repCs[TT]
        for e2 in range(2):
            T = 2 * TT + e2
            px = bass_ts(T, TPIX)
            o_ps = opool.tile([64, TPIX], F32)
            for grp in groups:
                gn = len(grp)
                v_ps = vpool.tile([128, GROUP_PAIRS * TPIX], F32)
                for e, j in enumerate(grp):
                    m = j // 2
                    if m < 4:
                        rtile, prow = repA, 32 * m
                    elif m < 8:
                        rtile, prow = repB, 32 * (m - 4)
                    else:
                        rtile, prow = repC, 0
                    rhs = rtile[
                        prow:prow + 32, e2 * 8 * 66:(e2 + 1) * 8 * 66
                    ].rearrange("p (r c) -> p r c", c=66)[:, :, 0:64]
                    nc.tensor.matmul(
                        v_ps[:, e * TPIX:(e + 1) * TPIX],
                        lh_sb[prow:prow + 32, j * 128:(j + 1) * 128],
                        rhs,
                        start=True,
                        stop=True,
                        tile_position=(prow, 0),
                    )
                vv = v_ps[:, :gn * TPIX]
                u_sb = wpool.tile([128, GROUP_PAIRS * TPIX], F32)
                nc.vector._custom_dve(
                    ops["frac_abs"],
                    out=u_sb[:, :gn * TPIX],
                    in0=vv,
                    s0=MAGIC,
                )
                c_sb = wpool.tile([128, GROUP_PAIRS * TPIX], F32)
                nc.scalar.activation(
                    c_sb[:, :gn * TPIX],
                    u_sb[:, :gn * TPIX],
                    AF.Sin,
                    scale=cst_n2pi[:],
                    bias=cst_hpi[:],
                )
                r_sb = wpool.tile([128, GROUP_PAIRS * TPIX], BF16)
                nc.vector._custom_dve(
                    ops["recip_ksub"],
                    out=r_sb[:, :gn * TPIX],
                    in0=c_sb[:, :gn * TPIX],
                    s0=RECIP_C0,
                    s1=RECIP_C1,
                    imm2=float(KCONST),
                )
                for e, j in enumerate(grp):
                    nc.tensor.matmul(
                        o_ps[:],
                        sm_sb[:, j * 64:(j + 1) * 64],
                        r_sb[:, e * TPIX:(e + 1) * TPIX],
                        start=(j == 0),
                        stop=(j == NPAIR - 1),
                    )
            o_sb = outp.tile([64, TPIX], F32)
            nc.scalar.copy(o_sb[:], o_ps[:])
            nc.sync.dma_start(dram["out"][:, px], o_sb[:])
    ctx.close()


def bass_ts(i, size):
    return slice(i * size, (i + 1) * size)


_COMPILED = {}


def _get_graph():
    if "nc" in _COMPILED:
        return _COMPILED["nc"]
    import concourse.bacc as bacc
    import concourse.tile as tile
    import concourse.mybir as mybir

    F32 = mybir.dt.float32
    nc = bacc.Bacc(
        "TRN2", target_bir_lowering=False, debug=False, num_devices=8
    )
    dram = {
        "x": nc.dram_tensor("x", [IN_C, L_], F32, kind="ExternalInput").ap(),
        "lhsT": nc.dram_tensor(
            "lhsT", [128, NPAIR * 128], F32, kind="ExternalInput"
        ).ap(),
        "smat": nc.dram_tensor(
            "smat", [128, NPAIR * 64], mybir.dt.bfloat16, kind="ExternalInput"
        ).ap(),
        "out": nc.dram_tensor(
            "out", [OUT_C, L_], F32, kind="ExternalOutput"
        ).ap(),
        **{
            f"repd{i}": nc.dram_tensor(
                f"repd{i}", [288, 16 * 66], F32, kind="Internal"
            ).ap()
            for i in range(4)
        },
    }
    with tile.TileContext(nc) as tc:
        _build(tc, dram)
    nc.compile()
    _COMPILED["nc"] = nc
    return nc


def _run(x, weight, morr_output_scale, trace=False):
    from concourse import bass_utils

    lhsT, smat = host_prep(weight, morr_output_scale)
    in_maps = [
        {
            "x": np.ascontiguousarray(x[b].reshape(IN_C, L_).astype(np.float32)),
            "lhsT": lhsT,
            "smat": smat,
        }
        for b in range(B_)
    ]
    nc = _get_graph()
    res = bass_utils.run_bass_kernel_spmd(
        nc, in_maps, core_ids=list(range(8)), trace=trace
    )
    out = np.stack([res.results[b]["out"].reshape(OUT_C, H_, W_) for b in range(B_)])
    return out.astype(np.float32), res


def kernel(x, weight, morr_output_scale):
    out, _ = _run(
        np.asarray(x), np.asarray(weight), np.asarray(morr_output_scale)
    )
    return out
